# revision 2
# baseline (speedup 1.0000x reference)
"""Haar DWT (2x2 stride-2 blocks) on 8 Trainium2 NeuronCores — v2.

Input x: (32, 3, 512, 512) f32. Outputs (ll, lh, hl, hh): each (32, 3, 256, 256).

Data parallel over batch: 4 images (12 channel-images of 512x512) per core.
I/O in bf16, host pre/post-transposed as in v1: x arrives partition-major
[128, 12ch, 4t, 512] with each row's columns deinterleaved into
[even(256) | odd(256)]; the single output tensor leaves as
[128, 12ch, 4t, 512] with cols [0:256) = ll|lh (by partition half) and
[256:512) = hl|hh.

v2 insight (from HW traces): the kernel is HBM-bound, and the DMA queues
round-robin per descriptor across rings, so loads and stores move
concurrently at ~430 GB/s combined — but only if both descriptor streams
stay dense for the whole window. v1 serialized them (loads 8.6-29.7us,
stores 22.3-42.8us at ~300 GB/s each). v2:

  - all 8 load DMAs are pre-issued back-to-back on the SP ring at kernel
    start (the whole 6.3 MB shard fits in SBUF), so the load stream is
    self-driving;
  - stores are issued on the otherwise-idle Pool/GpSimd ring the moment a
    chunk's exits finish; first/last chunks are single-channel so the store
    stream starts early and drains in small quanta;
  - single persistent SBUF tiles for x and the output (slice-level deps, no
    pool-rotation WAR semaphores).

Compute is v1's proven TensorE scheme, minus the DVE negation scratch: a
second stationary weight -W replaces the -E operand. Per 128-row tile t
(PSUM pair [128, 2, 512] f32 per half-channel, bufs=4):

  mm1 (start): bank = W x [E|O]          -> cols 0:256 = R(E), 256:512 = R(O)
  mm2 (acc):   bank[0:256]  += W  x O    -> R(E)+R(O) = ll|lh
  mm3 (stop):  bank[256:512] += -W x E   -> R(O)-R(E) = hl|hh

where R is the 128->128 row butterfly (rows 2m,2m+1 -> 0.5*sum at partition
m, 0.5*diff at partition 64+m). +-0.5 is exact in bf16 and PSUM accumulates
in f32, so the only roundings are the bf16 input/output quantizations.
Exits: ACT copies even tiles, DVE odd tiles, each into its own slice of the
persistent output tile.
"""

import sys

import numpy as np

if "/opt/trn_rl_repo" not in sys.path:
    sys.path.insert(0, "/opt/trn_rl_repo")

import ml_dtypes

from concourse import bacc, bass, mybir
from concourse import tile
from concourse.bass_utils import run_bass_kernel_spmd

N_CORES = 8
B, C, H, W = 32, 3, 512, 512
BPC = B // N_CORES  # images per core
NCH = BPC * C  # channel images per core (12)
P = 128  # SBUF partitions
NT = H // P  # 128-row tiles per channel (4)
HW_OUT = H // 2  # 256
NH = NT // 2  # half-channels per channel (2)

# channel-group sizes per load/store chunk: small first chunks start the
# store stream early; small last chunks drain it faster (8KB descriptors in
# the middle; both bigger 12KB and smaller 4KB descriptors measured worse)
CHUNKS = (1, 1, 2, 2, 2, 2, 1, 1)

_CACHE = {}


def _butterfly_weights():
    """W[k, m]: m<64 -> 0.5*(row 2m + row 2m+1); m>=64 -> 0.5*(row 2m'+1 - row 2m')."""
    w = np.zeros((P, P), dtype=np.float32)
    for m in range(64):
        w[2 * m, m] = 0.5
        w[2 * m + 1, m] = 0.5
        w[2 * m, 64 + m] = -0.5
        w[2 * m + 1, 64 + m] = 0.5
    return np.stack([w, -w], axis=1).astype(ml_dtypes.bfloat16)  # [P, 2, P]


def _build():
    nc = bacc.Bacc("TRN2", target_bir_lowering=False, debug=False)
    bf16 = mybir.dt.bfloat16
    f32 = mybir.dt.float32
    x = nc.dram_tensor("x", [P, NCH, NT, W], bf16, kind="ExternalInput")
    w = nc.dram_tensor("w", [P, 2, P], bf16, kind="ExternalInput")
    out = nc.dram_tensor("out", [P, NCH, NT, W], bf16, kind="ExternalOutput")
    scratch = nc.dram_tensor("scratch", [P, 64], bf16, kind="ExternalOutput")
    xa = x.ap()
    oa = out.ap()
    with tile.TileContext(nc) as tc:
        with (
            tc.tile_pool(name="p", bufs=1) as pool,
            tc.tile_pool(name="ps", bufs=4, space=bass.MemorySpace.PSUM) as psum,
        ):
            wt = pool.tile([P, 2, P], bf16)
            xin = pool.tile([P, NCH, NT, W], bf16)
            oT = pool.tile([P, NCH, NT, W], bf16)
            dummy = pool.tile([P, 64], bf16)
            # weights go FIRST on the fast SP ring: the first matmul is gated
            # on them (the Pool ring's first transfer lags its descriptor by
            # ~3us, which cost 2us of TensorE start in the previous rev)
            nc.sync.dma_start(out=wt[:], in_=w.ap())
            # pre-issue ALL loads: the SP ring descriptor stream covers the
            # whole 6.3 MB input up front, so the load side is self-driving
            c0 = 0
            for n in CHUNKS:
                nc.sync.dma_start(out=xin[:, c0 : c0 + n], in_=xa[:, c0 : c0 + n])
                c0 += n
            # warm the store (Pool) ring with a dummy transfer so the real
            # first store doesn't eat the ring's ~3-4us cold first-data lag
            nc.gpsimd.memset(dummy[:], 0)
            nc.gpsimd.dma_start(out=scratch.ap(), in_=dummy[:])
            c0 = 0
            for n in CHUNKS:
                for cc in range(c0, c0 + n):
                    for h in range(NH):
                        pt = psum.tile([P, 2, W], f32)
                        for tl in range(2):
                            t = 2 * h + tl
                            nc.tensor.matmul(
                                pt[:, tl, :],
                                wt[:, 0],
                                xin[:, cc, t, :],
                                start=True,
                                stop=False,
                            )
                            nc.tensor.matmul(
                                pt[:, tl, 0:HW_OUT],
                                wt[:, 0],
                                xin[:, cc, t, HW_OUT:W],
                                start=False,
                                stop=False,
                            )
                            nc.tensor.matmul(
                                pt[:, tl, HW_OUT:W],
                                wt[:, 1],
                                xin[:, cc, t, 0:HW_OUT],
                                start=False,
                                stop=True,
                            )
                        # exits: ACT takes the even tile, DVE the odd tile
                        nc.scalar.copy(oT[:, cc, 2 * h], pt[:, 0, :])
                        nc.vector.tensor_copy(oT[:, cc, 2 * h + 1], pt[:, 1, :])
                nc.gpsimd.dma_start(out=oa[:, c0 : c0 + n], in_=oT[:, c0 : c0 + n])
                c0 += n
    nc.compile()
    return nc


def _get_nc():
    if "nc" not in _CACHE:
        _CACHE["nc"] = _build()
    return _CACHE["nc"]


def run(x, **spmd_kwargs):
    """Run the DWT on 8 cores; returns (results_tuple, BassKernelResults)."""
    nc = _get_nc()
    xbf = np.ascontiguousarray(np.asarray(x, dtype=np.float32)).astype(
        ml_dtypes.bfloat16
    )
    # (B,C,H,W) -> [core, NCH, NT, P, j, eo] -> [core, P, NCH, NT, eo, j]
    xs = xbf.reshape(N_CORES, NCH, NT, P, HW_OUT, 2).transpose(0, 3, 1, 2, 5, 4)
    xs = np.ascontiguousarray(xs).reshape(N_CORES, P, NCH, NT, W)
    wmat = _butterfly_weights()
    in_maps = [{"x": xs[i], "w": wmat} for i in range(N_CORES)]
    res = None
    for attempt in range(3):
        try:
            res = run_bass_kernel_spmd(
                nc, in_maps, core_ids=list(range(N_CORES)), **spmd_kwargs
            )
            break
        except Exception:
            # transient device wedge (NRT_EXEC_UNIT_UNRECOVERABLE) recovers
            # on retry; re-raise only if it persists
            if attempt == 2:
                raise
            import time

            time.sleep(2)
    fo = np.stack([res.results[i]["out"] for i in range(N_CORES)])

    def expand(lo, j0):  # partitions [lo:lo+64), cols [j0:j0+256) -> (B,C,256,256)
        sl = fo[:, lo : lo + 64, :, :, j0 : j0 + HW_OUT].transpose(0, 2, 3, 1, 4)
        return np.ascontiguousarray(sl).astype(np.float32).reshape(B, C, HW_OUT, HW_OUT)

    ll = expand(0, 0)
    lh = expand(64, 0)
    hl = expand(0, HW_OUT)
    hh = expand(64, HW_OUT)
    return (ll, lh, hl, hh), res


def kernel(x):
    out, _ = run(x)
    return out
